# revision 19
# baseline (speedup 1.0000x reference)
"""Trainium2 Bass kernel for nn_Decoder (GRU decoder w/ Luong attention + big fc).

Strategy (8 NeuronCores):
- The sequential GRU phase is replicated on all cores (per-step collectives
  have a ~5us floor each - far too slow); everything runs in "transposed"
  layout [feature-on-partition, batch-on-free] so gate math uses 128 lanes.
- Phase B (attention, wa) replicated; the dominant fc matmul (67 GFLOP) is
  sharded over the vocab dim (4000 columns per core).
- log_softmax needs a global sum(exp(logits)) per row: one tiny (4KB)
  AllReduce; each core then writes its final fp32 output slice.
- Host side: embedding gather, transposes, bf16 casts, output concat.
"""
import numpy as np
import ml_dtypes

import concourse.bass as bass
import concourse.mybir as mybir
import concourse.tile as tile
from concourse.masks import make_identity

BF16 = ml_dtypes.bfloat16

B, T, H, E, S, V, NCORES = 32, 32, 1024, 512, 64, 32000, 8
VS = V // NCORES          # 4000 vocab cols per core
HC = H // 128             # 8 hidden chunks
MC = 3 * H // 128         # 24 gate-output chunks
EC = E // 128             # 4 embedding chunks
BT = B * T                # 1024 (row index bt = b*T + t)
TS = T + 1                # hs time slots (slot 0 = h0)
NV = VS // 500            # 8 vocab tiles of 500
FP32 = mybir.dt.float32
BF = mybir.dt.bfloat16
AX = mybir.AxisListType
AF = mybir.ActivationFunctionType


# ---------------------------------------------------------------------------
# Workarounds: this walrus build supports only ~2 sync waits per instruction.
# 1) split the tile-exit drain's waits onto single-wait SP nops;
# 2) post-pass any instruction carrying >2 waits.
def _patched_drain_and_barrier(self, tick_clock, wait_clock):
    from concourse.vector_clock import ScopedClock
    nc = self.nc
    probe = nc.sync.nop(nofuse=True, hint="drain_wait_probe")
    wait_clock.add_sem_waits(probe.ins, ScopedClock({None: tick_clock.global_clock}))
    si = probe.ins.sync_info
    waits = list(si.on_wait) if si is not None else []
    if len(waits) > 1:
        si.on_wait = [waits[0]]
        for w in waits[1:]:
            extra = nc.sync.nop(nofuse=True, hint="drain_wait_extra")
            esi = extra.ins.sync_info
            if esi is None:
                extra.ins.sync_info = mybir.SyncInfo(on_wait=[w], on_update=[])
            else:
                esi.on_wait = list(esi.on_wait) + [w]
    nc.sync.drain()
    nc.all_engine_barrier()
    assert self.sems is not None
    popped = nc._tile_sem_poison_stack.pop()
    assert popped is self._sem_poison
    nc.clear_and_free_semaphores(list(self.sems.allocated().values()))
    nc.all_engine_barrier()


tile.TileContext._drain_and_barrier = _patched_drain_and_barrier

MAX_WAITS = 1


def split_excess_waits(nc):
    n_split = 0
    for f in nc.m.functions:
        for bb in f.blocks:
            out = []
            for ins in bb.instructions:
                si = ins.sync_info
                if si is not None and len(si.on_wait) > MAX_WAITS:
                    waits = list(si.on_wait)
                    excess, keep = waits[:-MAX_WAITS], waits[-MAX_WAITS:]
                    for i in range(0, len(excess), MAX_WAITS):
                        n_split += 1
                        out.append(mybir.InstNoOp(
                            name=f"waitnop_{n_split}",
                            engine=ins.engine,
                            sync_info=mybir.SyncInfo(
                                on_wait=excess[i:i + MAX_WAITS], on_update=[]),
                        ))
                    si.on_wait = keep
                out.append(ins)
            bb.instructions[:] = out
    return n_split
# ---------------------------------------------------------------------------


def build_nc(t_steps=T, debug=False):
    nc = bass.Bass("TRN2", target_bir_lowering=False)

    # --- kernel I/O (per-core) ---
    whhT_d = nc.declare_dram_parameter("whhT", [H, 3 * H], BF, isOutput=False)
    wihT_d = nc.declare_dram_parameter("wihT", [E, 3 * H], BF, isOutput=False)
    xembT_d = nc.declare_dram_parameter("xembT", [E, BT], BF, isOutput=False)
    brz_d = nc.declare_dram_parameter("brz", [128, 16], FP32, isOutput=False)
    bin_d = nc.declare_dram_parameter("bin", [128, 8], FP32, isOutput=False)
    bhn_d = nc.declare_dram_parameter("bhn", [128, 8], FP32, isOutput=False)
    h0_d = nc.declare_dram_parameter("h0T", [128, 256], FP32, isOutput=False)
    encT_d = nc.declare_dram_parameter("encT", [H, B * S], BF, isOutput=False)
    encN_d = nc.declare_dram_parameter("encN", [B, S, H], BF, isOutput=False)
    waT_d = nc.declare_dram_parameter("waT", [2 * H, H], BF, isOutput=False)
    fcwT_d = nc.declare_dram_parameter("fcwT", [H, VS], BF, isOutput=False)
    fcb_d = nc.declare_dram_parameter("fcb", [1, VS], FP32, isOutput=False)
    out_d = nc.declare_dram_parameter("out", [BT, VS], FP32, isOutput=True)
    hlast_d = nc.declare_dram_parameter("hlast", [128, 256], FP32, isOutput=True)

    se_loc = nc.dram_tensor("se_loc", [128, 8], FP32)
    se_glob = nc.dram_tensor("se_glob", [128, 8], FP32, addr_space="Shared")

    if debug:
        dbg = {
            "hs_dump": nc.declare_dram_parameter(
                "hs_dump", [128, HC * B * TS], BF, isOutput=True),
            "sc_dump": nc.declare_dram_parameter(
                "sc_dump", [128, 512], FP32, isOutput=True),
            "wn_dump": nc.declare_dram_parameter(
                "wn_dump", [128, 512], BF, isOutput=True),
            "wT_dump": nc.declare_dram_parameter(
                "wT_dump", [64, 1024], BF, isOutput=True),
            "ctx_dump": nc.declare_dram_parameter(
                "ctx_dump", [128, HC * BT], BF, isOutput=True),
            "oT_dump": nc.declare_dram_parameter(
                "oT_dump", [128, HC * BT], BF, isOutput=True),
            "se_dump": nc.declare_dram_parameter(
                "se_dump", [128, 8], FP32, isOutput=True),
        }

    with tile.TileContext(nc) as tc:
        with (
            tc.tile_pool(name="persist", bufs=1) as pp,
            tc.tile_pool(name="state", bufs=2) as statep,
        ):
            hs = pp.tile([128, HC * B * TS], BF)          # col = c*1056 + b*33 + ts
            brz = pp.tile([128, 16], FP32)
            bin_ = pp.tile([128, 8], FP32)
            bhn = pp.tile([128, 8], FP32)
            nc.sync.dma_start(brz[:], brz_d[:])
            nc.sync.dma_start(bin_[:], bin_d[:])
            nc.sync.dma_start(bhn[:], bhn_d[:])

            hsv = hs[:].rearrange("p (c b s) -> p c b s", c=HC, b=B, s=TS)

            h_f = statep.tile([128, 256], FP32, tag="hf")
            nc.sync.dma_start(h_f[:], h0_d[:])
            nc.scalar.copy(hsv[:, :, :, 0],
                           h_f[:].rearrange("p (c b) -> p c b", c=HC))

            # ====== gi = x_emb @ w_ih.T (+ folded biases), then GRU loop ======
            with (
                tc.tile_pool(name="giwh", bufs=1) as gwp,
                tc.tile_pool(name="gi_tr", bufs=1) as gtp,
                tc.tile_pool(name="ga", bufs=2) as gap,
                tc.tile_pool(name="a_ps", bufs=2, space="PSUM") as apsp,
            ):
                gi = gwp.tile([128, MC * BT], BF)         # col = m*1024 + t*32 + b
                giv = gi[:].rearrange("p (m t b) -> p m t b", m=MC, t=T, b=B)

                _sid_gi = nc.enter_named_scope("gi", False)[0]
                xe = gtp.tile([128, EC * BT], BF)
                for k in range(EC):
                    nc.sync.dma_start(xe[:, k * BT:(k + 1) * BT],
                                      xembT_d[k * 128:(k + 1) * 128, :])
                wih = [gtp.tile([128, 3 * H], BF, tag=f"wih{k}", name=f"wih{k}")
                       for k in range(EC)]
                for k in range(EC):
                    nc.sync.dma_start(wih[k][:], wihT_d[k * 128:(k + 1) * 128, :])
                for m in range(MC):
                    for n in range(2):
                        ps = apsp.tile([128, 512], FP32, tag="gips")
                        for k in range(EC):
                            nc.tensor.matmul(
                                ps[:],
                                wih[k][:, m * 128:(m + 1) * 128],
                                xe[:, k * BT + n * 512: k * BT + (n + 1) * 512],
                                start=(k == 0), stop=(k == EC - 1),
                            )
                        bias = brz[:, m:m + 1] if m < 16 else bin_[:, m - 16:m - 15]
                        nc.scalar.activation(
                            gi[:, m * BT + n * 512: m * BT + (n + 1) * 512],
                            ps[:], AF.Identity, bias=bias, scale=1.0)

                nc.leave_named_scope("gi", _sid_gi, False)
                # ---- phase A: GRU recurrence (replicated) ----
                _sid_pa = nc.enter_named_scope("phaseA", False)[0]
                whh = gwp.tile([128, HC * 3 * H], BF)
                for k in range(HC):
                    nc.sync.dma_start(whh[:, k * 3 * H:(k + 1) * 3 * H],
                                      whhT_d[k * 128:(k + 1) * 128, :])
                for t in range(t_steps):
                    ps = apsp.tile([128, 768], FP32, tag="gh")
                    for m in range(MC):
                        for k in range(HC):
                            nc.tensor.matmul(
                                ps[:, m * 32:(m + 1) * 32],
                                whh[:, k * 3 * H + m * 128: k * 3 * H + (m + 1) * 128],
                                hsv[:, k, :, t],
                                start=(k == 0), stop=(k == HC - 1),
                            )
                    psv = ps[:].rearrange("p (m b) -> p m b", b=32)
                    tmp_rz = gap.tile([128, 512], FP32, tag="trz")
                    nc.vector.tensor_add(
                        tmp_rz[:].rearrange("p (m b) -> p m b", b=32),
                        psv[:, 0:16, :], giv[:, 0:16, t, :])
                    rz = gap.tile([128, 512], FP32, tag="rz")
                    nc.scalar.activation(rz[:], tmp_rz[:], AF.Sigmoid)
                    hn = gap.tile([128, 256], FP32, tag="hn")
                    for c in range(HC):
                        nc.vector.tensor_scalar_add(
                            hn[:, c * 32:(c + 1) * 32],
                            psv[:, 16 + c, :], bhn[:, c:c + 1])
                    rhn = gap.tile([128, 256], FP32, tag="rhn")
                    nc.vector.tensor_mul(rhn[:], rz[:, 0:256], hn[:])
                    t5 = gap.tile([128, 256], FP32, tag="t5")
                    nc.vector.tensor_add(
                        t5[:].rearrange("p (m b) -> p m b", b=32),
                        rhn[:].rearrange("p (m b) -> p m b", b=32),
                        giv[:, 16:24, t, :])
                    nt = gap.tile([128, 256], FP32, tag="nt")
                    nc.scalar.activation(nt[:], t5[:], AF.Tanh)
                    dh = gap.tile([128, 256], FP32, tag="dh")
                    nc.vector.tensor_sub(dh[:], h_f[:], nt[:])
                    zdh = gap.tile([128, 256], FP32, tag="zdh")
                    nc.vector.tensor_mul(zdh[:], rz[:, 256:512], dh[:])
                    h_new = statep.tile([128, 256], FP32, tag="hf")
                    nc.vector.tensor_add(h_new[:], nt[:], zdh[:])
                    nc.scalar.copy(hsv[:, :, :, t + 1],
                                   h_new[:].rearrange("p (c b) -> p c b", c=HC))
                    h_f = h_new
                nc.leave_named_scope("phaseA", _sid_pa, False)
            nc.sync.dma_start(hlast_d[:], h_f[:])
            if debug:
                nc.sync.dma_start(dbg["hs_dump"][:], hs[:])

            # ================= attention =================
            ctxT = pp.tile([128, HC * BT], BF)            # col = c*1024 + b*32 + t
            ctv = ctxT[:].rearrange("p (c b t) -> p c b t", c=HC, b=B, t=T)
            with (
                tc.tile_pool(name="attn", bufs=2) as ap_,
                tc.tile_pool(name="attn1", bufs=1) as ap1,
                tc.tile_pool(name="at_ps", bufs=1, space="PSUM") as psp,
                tc.tile_pool(name="at_ps2", bufs=2, space="PSUM") as psp2,
            ):
                _sid_at = nc.enter_named_scope("attn", False)[0]
                ident = ap1.tile([128, 128], BF)
                make_identity(nc, ident[:])
                # scores [t, s] per b; col-tiled 4 b's concurrently
                sc_ps = psp.tile([128, 512], FP32, tag="sc")
                for c in range(HC):
                    encT_t = ap_.tile([128, B * S], BF, tag="encT")
                    nc.sync.dma_start(encT_t[:], encT_d[c * 128:(c + 1) * 128, :])
                    for b in range(B):
                        g, j = b // 4, b % 4
                        # NB: start=True clears has_written for the whole
                        # bank within this partition strip, so only the
                        # FIRST b per strip may set it.
                        nc.tensor.matmul(
                            sc_ps[32 * j:32 * (j + 1), g * 64:(g + 1) * 64],
                            hsv[:, c, b, 1:TS],
                            encT_t[:, b * S:(b + 1) * S],
                            start=(c == 0 and b < 4), stop=(c == HC - 1),
                            tile_position=(0, 32 * j),
                            skip_group_check=True,
                        )
                if debug:
                    sc_sb = ap1.tile([128, 512], FP32)
                    nc.vector.tensor_copy(sc_sb[:], sc_ps[:])
                    nc.sync.dma_start(dbg["sc_dump"][:], sc_sb[:])
                scv = sc_ps[:].rearrange("p (g s) -> p g s", g=8)
                mx = ap1.tile([128, 8], FP32)
                nc.vector.reduce_max(
                    mx[:].rearrange("p (g o) -> p g o", o=1), scv, axis=AX.X)
                nmx = ap1.tile([128, 8], FP32)
                nc.vector.tensor_scalar_mul(nmx[:], mx[:], -1.0)
                wexp = ap1.tile([128, 512], BF)
                zs = ap1.tile([128, 8], FP32)
                for g in range(8):
                    nc.scalar.activation(
                        wexp[:, g * 64:(g + 1) * 64], sc_ps[:, g * 64:(g + 1) * 64],
                        AF.Exp, bias=nmx[:, g:g + 1], scale=1.0,
                        accum_out=zs[:, g:g + 1])
                rzs = ap1.tile([128, 8], FP32)
                nc.vector.reciprocal(rzs[:], zs[:])
                wn = ap1.tile([128, 512], BF)
                for g in range(8):
                    nc.vector.tensor_scalar_mul(
                        wn[:, g * 64:(g + 1) * 64],
                        wexp[:, g * 64:(g + 1) * 64], rzs[:, g:g + 1])
                # transpose w -> wT [64 (s), g*128 + j*32 + t]
                wT_ps = psp.tile([64, 1024], BF, tag="wt")
                for g in range(8):
                    nc.tensor.transpose(
                        wT_ps[:, g * 128:(g + 1) * 128],
                        wn[:, g * 64:(g + 1) * 64], ident[:])
                wT = ap1.tile([64, 1024], BF)
                nc.vector.tensor_copy(wT[:], wT_ps[:])
                if debug:
                    nc.sync.dma_start(dbg["wn_dump"][:], wn[:])
                    nc.sync.dma_start(dbg["wT_dump"][:], wT[:])

                # ctx[b] = enc_b.T @ w_b; 16 b's batched per psum flush
                for half in range(2):
                    encbs = ap_.tile([64, 16 * H], BF, tag="encbs")
                    for bl in range(16):
                        nc.sync.dma_start(
                            encbs[:, bl * H:(bl + 1) * H],
                            encN_d[half * 16 + bl, :, :])
                    for c in range(HC):
                        ctx_ps = psp2.tile([128, 512], FP32, tag="cx")
                        for bl in range(16):
                            b = half * 16 + bl
                            g, j = b // 4, b % 4
                            nc.tensor.matmul(
                                ctx_ps[:, bl * 32:(bl + 1) * 32],
                                encbs[:, bl * H + c * 128: bl * H + (c + 1) * 128],
                                wT[:, g * 128 + j * 32: g * 128 + (j + 1) * 32],
                                start=True, stop=True,
                                skip_group_check=True,
                            )
                        nc.vector.tensor_copy(
                            ctv[:, c, half * 16:(half + 1) * 16, :],
                            ctx_ps[:].rearrange("p (b t) -> p b t", t=32))

                nc.leave_named_scope("attn", _sid_at, False)
            if debug:
                nc.sync.dma_start(dbg["ctx_dump"][:], ctxT[:])

            # ================= o = tanh([hs; ctx] @ wa.T) =================
            oT = pp.tile([128, HC * BT], BF)              # col = m*1024 + bt
            with (
                tc.tile_pool(name="wa", bufs=1) as wap,
                tc.tile_pool(name="wa_ps", bufs=2, space="PSUM") as wpsp,
            ):
                _sid_wa = nc.enter_named_scope("wa", False)[0]
                wa_sb = wap.tile([128, 16 * H], BF)
                for k in range(16):
                    nc.sync.dma_start(wa_sb[:, k * H:(k + 1) * H],
                                      waT_d[k * 128:(k + 1) * 128, :])
                for m in range(HC):
                    for n in range(2):
                        ps = wpsp.tile([128, 512], FP32, tag="wps")
                        for k in range(16):
                            if k < HC:
                                rhs = hsv[:, k, n * 16:(n + 1) * 16, 1:TS]
                            else:
                                rhs = ctv[:, k - HC, n * 16:(n + 1) * 16, :]
                            nc.tensor.matmul(
                                ps[:], wa_sb[:, k * H + m * 128: k * H + (m + 1) * 128],
                                rhs, start=(k == 0), stop=(k == 15),
                            )
                        nc.scalar.activation(
                            oT[:, m * BT + n * 512: m * BT + (n + 1) * 512],
                            ps[:], AF.Tanh)
                nc.leave_named_scope("wa", _sid_wa, False)

            # ================= fc + log_softmax =================
            with (
                tc.tile_pool(name="fc", bufs=2) as fcp,
                tc.tile_pool(name="fc1", bufs=1) as fc1,
                tc.tile_pool(name="fc_ps", bufs=2, space="PSUM") as fpsp,
            ):
                _sid_fc = nc.enter_named_scope("fc", False)[0]
                logits = fc1.tile([128, HC * VS], BF)     # col = j*4000 + n*500
                fcb_sb = fc1.tile([1, VS], FP32)
                nc.sync.dma_start(fcb_sb[:], fcb_d[:])
                ones = fc1.tile([1, 128], FP32)
                nc.vector.memset(ones[:], 1.0)
                fcb_bc = fc1.tile([128, VS], BF)
                for n in range(NV):
                    psb = fpsp.tile([128, 500], FP32, tag="fcb")
                    nc.tensor.matmul(psb[:], ones[:],
                                     fcb_sb[:, n * 500:(n + 1) * 500],
                                     start=True, stop=True)
                    nc.vector.tensor_copy(fcb_bc[:, n * 500:(n + 1) * 500], psb[:])

                se_acc = fc1.tile([128, 8], FP32)
                nc.vector.memset(se_acc[:], 0.0)
                for n in range(NV):
                    fwt = fcp.tile([128, HC * 500], BF, tag="fwt")
                    for k in range(HC):
                        nc.sync.dma_start(
                            fwt[:, k * 500:(k + 1) * 500],
                            fcwT_d[k * 128:(k + 1) * 128, n * 500:(n + 1) * 500])
                    for j in range(HC):
                        ps = fpsp.tile([128, 500], FP32, tag="fps")
                        for k in range(HC):
                            nc.tensor.matmul(
                                ps[:],
                                oT[:, k * BT + j * 128: k * BT + (j + 1) * 128],
                                fwt[:, k * 500:(k + 1) * 500],
                                start=(k == 0), stop=(k == HC - 1),
                            )
                        lslice = logits[:, j * VS + n * 500: j * VS + (n + 1) * 500]
                        nc.vector.tensor_add(lslice, ps[:],
                                             fcb_bc[:, n * 500:(n + 1) * 500])
                        escr = fcp.tile([128, 500], FP32, tag="escr")
                        pe = fcp.tile([128, 1], FP32, tag="pe")
                        nc.scalar.activation(escr[:], lslice, AF.Exp,
                                             accum_out=pe[:])
                        nc.vector.tensor_add(se_acc[:, j:j + 1],
                                             se_acc[:, j:j + 1], pe[:])

                if debug:
                    nc.sync.dma_start(dbg["oT_dump"][:], oT[:])
                    nc.sync.dma_start(dbg["se_dump"][:], se_acc[:])

                # global sum(exp) across vocab shards
                nc.sync.dma_start(se_loc[:], se_acc[:])
                nc.gpsimd.collective_compute(
                    "AllReduce", mybir.AluOpType.add,
                    ins=[se_loc[:]], outs=[se_glob[:]],
                    replica_groups=[list(range(NCORES))],
                )
                se_g = fc1.tile([128, 8], FP32)
                nc.sync.dma_start(se_g[:], se_glob[:])
                lse = fc1.tile([128, 8], FP32)
                nc.scalar.activation(lse[:], se_g[:], AF.Ln)
                nlse = fc1.tile([128, 8], FP32)
                nc.vector.tensor_scalar_mul(nlse[:], lse[:], -1.0)
                nc.leave_named_scope("fc", _sid_fc, False)

                _sid_ow = nc.enter_named_scope("outw", False)[0]
                for j in range(HC):
                    for n in range(NV):
                        ot = fcp.tile([128, 500], FP32, tag="fout")
                        nc.scalar.activation(
                            ot[:],
                            logits[:, j * VS + n * 500: j * VS + (n + 1) * 500],
                            AF.Identity, bias=nlse[:, j:j + 1], scale=1.0)
                        nc.sync.dma_start(
                            out_d[j * 128:(j + 1) * 128, n * 500:(n + 1) * 500],
                            ot[:])
                nc.leave_named_scope("outw", _sid_ow, False)
    split_excess_waits(nc)
    return nc


_NC_CACHE = {}


def _get_nc(t_steps=T, debug=False):
    key = (t_steps, debug)
    if key not in _NC_CACHE:
        _NC_CACHE[key] = build_nc(t_steps, debug)
    return _NC_CACHE[key]


def host_prep(inputs):
    """Build the per-core input maps from the full inputs."""
    target = np.asarray(inputs["target"])
    enc_h = np.asarray(inputs["encoder_hidden"], np.float32)
    enc_o = np.asarray(inputs["encoder_outputs"], np.float32)
    emb = np.asarray(inputs["emb"], np.float32)
    w_ih = np.asarray(inputs["w_ih"], np.float32)
    w_hh = np.asarray(inputs["w_hh"], np.float32)
    b_ih = np.asarray(inputs["b_ih"], np.float32)
    b_hh = np.asarray(inputs["b_hh"], np.float32)
    wa = np.asarray(inputs["wa"], np.float32)
    fc_w = np.asarray(inputs["fc_w"], np.float32)
    fc_b = np.asarray(inputs["fc_b"], np.float32)

    in_ids = np.concatenate(
        [np.full((B, 1), 1, target.dtype), target[:, :T - 1]], axis=1)
    x_emb = emb[in_ids]                               # [B,T,E]
    xembT = np.ascontiguousarray(
        x_emb.transpose(2, 1, 0).reshape(E, BT)).astype(BF16)  # col = t*32+b

    whhT = np.ascontiguousarray(w_hh.T).astype(BF16)  # [H, 3H]
    wihT = np.ascontiguousarray(w_ih.T).astype(BF16)  # [E, 3H]

    brz_v = (b_ih[:2 * H] + b_hh[:2 * H]).reshape(16, 128).T.copy()   # [128,16]
    bin_v = b_ih[2 * H:].reshape(8, 128).T.copy()                     # [128,8]
    bhn_v = b_hh[2 * H:].reshape(8, 128).T.copy()                     # [128,8]

    h0 = enc_h[0]                                      # [B,H]
    h0T = np.ascontiguousarray(
        h0.T.reshape(HC, 128, B).transpose(1, 0, 2).reshape(128, 256))

    encT = np.ascontiguousarray(
        enc_o.transpose(2, 0, 1).reshape(H, B * S)).astype(BF16)
    encN = enc_o.astype(BF16)
    waT = np.ascontiguousarray(wa.T).astype(BF16)      # [2H, H]

    common = {
        "whhT": whhT, "wihT": wihT, "xembT": xembT,
        "brz": brz_v.astype(np.float32), "bin": bin_v.astype(np.float32),
        "bhn": bhn_v.astype(np.float32), "h0T": h0T.astype(np.float32),
        "encT": encT, "encN": encN, "waT": waT,
    }
    in_maps = []
    for c in range(NCORES):
        v0, v1 = c * VS, (c + 1) * VS
        m = dict(common)
        m["fcwT"] = np.ascontiguousarray(fc_w[v0:v1].T).astype(BF16)
        m["fcb"] = fc_b[v0:v1].reshape(1, VS).astype(np.float32)
        in_maps.append(m)
    return in_maps


def run_spmd(in_maps, t_steps=T, trace=False, debug=False, **kw):
    from concourse.bass_utils import run_bass_kernel_spmd
    nc = _get_nc(t_steps, debug)
    return run_bass_kernel_spmd(nc, in_maps, list(range(NCORES)), trace=trace, **kw)


def assemble(results):
    out_full = np.concatenate([results[c]["out"] for c in range(NCORES)], axis=1)
    decoder_outputs = out_full.reshape(B, T, V)
    hl = results[0]["hlast"]                            # [128, 256]
    h_last = hl.reshape(128, HC, B).transpose(2, 1, 0).reshape(B, H)
    return decoder_outputs, h_last[None]


def kernel(**inputs):
    in_maps = host_prep(inputs)
    res = run_spmd(in_maps)
    return assemble(res.results)


# revision 25
# speedup vs baseline: 1.0698x; 1.0698x over previous
"""Trainium2 Bass kernel for nn_Decoder (GRU decoder w/ Luong attention + big fc).

Strategy (8 NeuronCores):
- The sequential GRU phase is replicated on all cores (per-step collectives
  have a ~5us floor each - far too slow); everything runs in "transposed"
  layout [feature-on-partition, batch-on-free] so gate math uses 128 lanes.
- Phase B (attention, wa) replicated; the dominant fc matmul (67 GFLOP) is
  sharded over the vocab dim (4000 columns per core).
- log_softmax needs a global sum(exp(logits)) per row: one tiny (4KB)
  AllReduce; each core then writes its final fp32 output slice.
- Host side: embedding gather, transposes, bf16 casts, output concat.
"""
import numpy as np
import ml_dtypes

import concourse.bass as bass
import concourse.mybir as mybir
import concourse.tile as tile
from concourse.masks import make_identity

BF16 = ml_dtypes.bfloat16

B, T, H, E, S, V, NCORES = 32, 32, 1024, 512, 64, 32000, 8
VS = V // NCORES          # 4000 vocab cols per core
HC = H // 128             # 8 hidden chunks
MC = 3 * H // 128         # 24 gate-output chunks
EC = E // 128             # 4 embedding chunks
BT = B * T                # 1024 (row index bt = b*T + t)
TS = T + 1                # hs time slots (slot 0 = h0)
NV = VS // 500            # 8 vocab tiles of 500
FP32 = mybir.dt.float32
BF = mybir.dt.bfloat16
AX = mybir.AxisListType
AF = mybir.ActivationFunctionType


# ---------------------------------------------------------------------------
# Workarounds: this walrus build supports only ~2 sync waits per instruction.
# 1) split the tile-exit drain's waits onto single-wait SP nops;
# 2) post-pass any instruction carrying >2 waits.
def _patched_drain_and_barrier(self, tick_clock, wait_clock):
    from concourse.vector_clock import ScopedClock
    nc = self.nc
    probe = nc.sync.nop(nofuse=True, hint="drain_wait_probe")
    wait_clock.add_sem_waits(probe.ins, ScopedClock({None: tick_clock.global_clock}))
    si = probe.ins.sync_info
    waits = list(si.on_wait) if si is not None else []
    if len(waits) > 1:
        si.on_wait = [waits[0]]
        for w in waits[1:]:
            extra = nc.sync.nop(nofuse=True, hint="drain_wait_extra")
            esi = extra.ins.sync_info
            if esi is None:
                extra.ins.sync_info = mybir.SyncInfo(on_wait=[w], on_update=[])
            else:
                esi.on_wait = list(esi.on_wait) + [w]
    nc.sync.drain()
    nc.all_engine_barrier()
    assert self.sems is not None
    popped = nc._tile_sem_poison_stack.pop()
    assert popped is self._sem_poison
    nc.clear_and_free_semaphores(list(self.sems.allocated().values()))
    nc.all_engine_barrier()


tile.TileContext._drain_and_barrier = _patched_drain_and_barrier


# Enable walrus' fast-weight-load codegen (2x LDWEIGHTS for bf16, 4x for
# fp8); concourse pins it off, but this kernel is LDWEIGHTS-bound in the
# GRU phase. Correctness is validated against the reference by the harness.
def _patch_ldw_opt():
    import concourse.bass_utils as _bu
    if getattr(_bu, "_ldw_opt_patched", False):
        return
    _orig = _bu.run_command

    def _run_command_fwl(cmd, **kw):
        cmd = [("--enable-ldw-opt=true" if c == "--enable-ldw-opt=false" else c)
               for c in cmd]
        return _orig(cmd, **kw)

    _bu.run_command = _run_command_fwl
    _bu._ldw_opt_patched = True


MAX_WAITS = 1


def fuse_ldweights(nc):
    """Delete Tile's standalone InstLdweights (the paired InstMatmult still
    carries the weights operand), moving their sem waits/updates onto the
    matmul. Leaves pure self-loading matmuls, which walrus' LDW optimization
    (fast weight load) accepts."""
    import collections
    n = 0
    for f in nc.m.functions:
        for bb in f.blocks:
            out = []
            pending = collections.deque()
            for ins in bb.instructions:
                tn = type(ins).__name__
                if tn == "InstLdweights":
                    pending.append(ins)
                    continue
                if tn == "InstMatmult" and pending:
                    ldw = pending.popleft()
                    lw = ldw.ins[0]
                    mw = ins.ins[1]
                    assert (lw.memref == mw.memref
                            and lw.offset == mw.offset), (
                        f"LDW/MM mismatch {lw.memref}@{lw.offset} vs "
                        f"{mw.memref}@{mw.offset}")
                    psi = ldw.sync_info
                    if psi is not None and (psi.on_wait or psi.on_update):
                        si = ins.sync_info
                        if si is None:
                            ins.sync_info = mybir.SyncInfo(
                                on_wait=list(psi.on_wait),
                                on_update=list(psi.on_update))
                        else:
                            si.on_wait = list(psi.on_wait) + list(si.on_wait)
                            si.on_update = (list(psi.on_update)
                                            + list(si.on_update))
                    n += 1
                out.append(ins)
            assert not pending, f"dangling LDWs in {bb.name}"
            bb.instructions[:] = out
    return n


def dedupe_ldweights(nc):
    """Remove an InstLdweights identical to the immediately-preceding one on
    the PE stream (same weights AP + tile_position): the array still holds
    those weights, and the matmul's own weights operand is not re-loaded by
    codegen when a matching LDW precedes it. Waits move to the next PE inst."""
    n = 0
    for f in nc.m.functions:
        for bb in f.blocks:
            out = []
            last_sig = None
            pend_waits = []
            for ins in bb.instructions:
                tn = type(ins).__name__
                if tn == "InstLdweights":
                    a = ins.ins[0]
                    sig = (a.memref, a.offset, str(a.ap),
                           getattr(ins, "tile_position", None),
                           getattr(ins, "is_transpose", None))
                    if sig == last_sig:
                        si = ins.sync_info
                        if si is not None:
                            pend_waits += list(si.on_wait)
                            assert not si.on_update
                        n += 1
                        continue
                    last_sig = sig
                elif tn == "InstMatmult":
                    if pend_waits:
                        si = ins.sync_info
                        if si is None:
                            ins.sync_info = mybir.SyncInfo(
                                on_wait=pend_waits, on_update=[])
                        else:
                            si.on_wait = pend_waits + list(si.on_wait)
                        pend_waits = []
                out.append(ins)
            assert not pend_waits
            bb.instructions[:] = out
    return n


def split_excess_waits(nc):
    n_split = 0
    for f in nc.m.functions:
        for bb in f.blocks:
            out = []
            for ins in bb.instructions:
                si = ins.sync_info
                if si is not None and len(si.on_wait) > MAX_WAITS:
                    waits = list(si.on_wait)
                    excess, keep = waits[:-MAX_WAITS], waits[-MAX_WAITS:]
                    for i in range(0, len(excess), MAX_WAITS):
                        n_split += 1
                        out.append(mybir.InstNoOp(
                            name=f"waitnop_{n_split}",
                            engine=ins.engine,
                            sync_info=mybir.SyncInfo(
                                on_wait=excess[i:i + MAX_WAITS], on_update=[]),
                        ))
                    si.on_wait = keep
                out.append(ins)
            bb.instructions[:] = out
    return n_split
# ---------------------------------------------------------------------------


def build_nc(t_steps=T, debug=False):
    nc = bass.Bass("TRN2", target_bir_lowering=False)

    # --- kernel I/O (per-core) ---
    whhT_d = nc.declare_dram_parameter("whhT", [H, 3 * H], BF, isOutput=False)
    wihT_d = nc.declare_dram_parameter("wihT", [E, 3 * H], BF, isOutput=False)
    xembT_d = nc.declare_dram_parameter("xembT", [E, BT], BF, isOutput=False)
    brz_d = nc.declare_dram_parameter("brz", [128, 16], FP32, isOutput=False)
    bin_d = nc.declare_dram_parameter("bin", [128, 8], FP32, isOutput=False)
    bhn_d = nc.declare_dram_parameter("bhn", [128, 8], FP32, isOutput=False)
    h0_d = nc.declare_dram_parameter("h0T", [128, 256], FP32, isOutput=False)
    encT_d = nc.declare_dram_parameter("encT", [H, B * S], BF, isOutput=False)
    encN_d = nc.declare_dram_parameter("encN", [B, S, H], BF, isOutput=False)
    waT_d = nc.declare_dram_parameter("waT", [2 * H, H], BF, isOutput=False)
    fcwT_d = nc.declare_dram_parameter("fcwT", [H, VS], BF, isOutput=False)
    fcb_d = nc.declare_dram_parameter("fcb", [1, VS], FP32, isOutput=False)
    out_d = nc.declare_dram_parameter("out", [BT, VS], FP32, isOutput=True)
    hlast_d = nc.declare_dram_parameter("hlast", [128, 256], FP32, isOutput=True)

    se_loc = nc.dram_tensor("se_loc", [128, 8], FP32)
    se_glob = nc.dram_tensor("se_glob", [128, 8], FP32, addr_space="Shared")

    if debug:
        dbg = {
            "hs_dump": nc.declare_dram_parameter(
                "hs_dump", [128, HC * B * TS], BF, isOutput=True),
            "sc_dump": nc.declare_dram_parameter(
                "sc_dump", [128, 512], FP32, isOutput=True),
            "wn_dump": nc.declare_dram_parameter(
                "wn_dump", [128, 512], BF, isOutput=True),
            "wT_dump": nc.declare_dram_parameter(
                "wT_dump", [64, 1024], BF, isOutput=True),
            "ctx_dump": nc.declare_dram_parameter(
                "ctx_dump", [128, HC * BT], BF, isOutput=True),
            "oT_dump": nc.declare_dram_parameter(
                "oT_dump", [128, HC * BT], BF, isOutput=True),
            "se_dump": nc.declare_dram_parameter(
                "se_dump", [128, 8], FP32, isOutput=True),
        }

    with tile.TileContext(nc) as tc:
        with (
            tc.tile_pool(name="persist", bufs=1) as pp,
            tc.tile_pool(name="state", bufs=2) as statep,
        ):
            hs = pp.tile([128, HC * B * TS], BF)          # col = c*1056 + b*33 + ts
            brz = pp.tile([128, 16], FP32)
            bin_ = pp.tile([128, 8], FP32)
            bhn = pp.tile([128, 8], FP32)
            nc.sync.dma_start(brz[:], brz_d[:])
            nc.sync.dma_start(bin_[:], bin_d[:])
            nc.sync.dma_start(bhn[:], bhn_d[:])

            hsv = hs[:].rearrange("p (c b s) -> p c b s", c=HC, b=B, s=TS)

            h_f = statep.tile([128, 256], FP32, tag="hf")
            nc.sync.dma_start(h_f[:], h0_d[:])
            nc.scalar.copy(hsv[:, :, :, 0],
                           h_f[:].rearrange("p (c b) -> p c b", c=HC))

            # ====== gi = x_emb @ w_ih.T (+ folded biases), then GRU loop ======
            with (
                tc.tile_pool(name="giwh", bufs=1) as gwp,
                tc.tile_pool(name="gi_tr", bufs=1) as gtp,
                tc.tile_pool(name="ga", bufs=2) as gap,
                tc.tile_pool(name="a_ps", bufs=2, space="PSUM") as apsp,
            ):
                gi = gwp.tile([128, MC * BT], BF)         # col = m*1024 + t*32 + b
                giv = gi[:].rearrange("p (m t b) -> p m t b", m=MC, t=T, b=B)

                _sid_gi = nc.enter_named_scope("gi", False)[0]
                xe = gtp.tile([128, EC * BT], BF)
                for k in range(EC):
                    nc.sync.dma_start(xe[:, k * BT:(k + 1) * BT],
                                      xembT_d[k * 128:(k + 1) * 128, :])
                wih = [gtp.tile([128, 3 * H], BF, tag=f"wih{k}", name=f"wih{k}")
                       for k in range(EC)]
                for k in range(EC):
                    nc.sync.dma_start(wih[k][:], wihT_d[k * 128:(k + 1) * 128, :])
                for m in range(MC):
                    for n in range(2):
                        ps = apsp.tile([128, 512], FP32, tag="gips")
                        for k in range(EC):
                            nc.tensor.matmul(
                                ps[:],
                                wih[k][:, m * 128:(m + 1) * 128],
                                xe[:, k * BT + n * 512: k * BT + (n + 1) * 512],
                                start=(k == 0), stop=(k == EC - 1),
                            )
                        bias = brz[:, m:m + 1] if m < 16 else bin_[:, m - 16:m - 15]
                        nc.scalar.activation(
                            gi[:, m * BT + n * 512: m * BT + (n + 1) * 512],
                            ps[:], AF.Identity, bias=bias, scale=1.0)

                nc.leave_named_scope("gi", _sid_gi, False)
                # ---- phase A: GRU recurrence (replicated) ----
                _sid_pa = nc.enter_named_scope("phaseA", False)[0]
                whh = gwp.tile([128, HC * 3 * H], BF)
                for k in range(HC):
                    nc.sync.dma_start(whh[:, k * 3 * H:(k + 1) * 3 * H],
                                      whhT_d[k * 128:(k + 1) * 128, :])
                for t in range(t_steps):
                    ps = apsp.tile([128, 768], FP32, tag="gh")
                    for m in range(MC):
                        for k in range(HC):
                            nc.tensor.matmul(
                                ps[:, m * 32:(m + 1) * 32],
                                whh[:, k * 3 * H + m * 128: k * 3 * H + (m + 1) * 128],
                                hsv[:, k, :, t],
                                start=(k == 0), stop=(k == HC - 1),
                            )
                    psv = ps[:].rearrange("p (m b) -> p m b", b=32)
                    tmp_rz = gap.tile([128, 512], FP32, tag="trz")
                    nc.vector.tensor_add(
                        tmp_rz[:].rearrange("p (m b) -> p m b", b=32),
                        psv[:, 0:16, :], giv[:, 0:16, t, :])
                    rz = gap.tile([128, 512], FP32, tag="rz")
                    nc.scalar.activation(rz[:], tmp_rz[:], AF.Sigmoid)
                    hn = gap.tile([128, 256], FP32, tag="hn")
                    for c in range(HC):
                        nc.vector.tensor_scalar_add(
                            hn[:, c * 32:(c + 1) * 32],
                            psv[:, 16 + c, :], bhn[:, c:c + 1])
                    rhn = gap.tile([128, 256], FP32, tag="rhn")
                    nc.vector.tensor_mul(rhn[:], rz[:, 0:256], hn[:])
                    t5 = gap.tile([128, 256], FP32, tag="t5")
                    nc.vector.tensor_add(
                        t5[:].rearrange("p (m b) -> p m b", b=32),
                        rhn[:].rearrange("p (m b) -> p m b", b=32),
                        giv[:, 16:24, t, :])
                    nt = gap.tile([128, 256], FP32, tag="nt")
                    nc.scalar.activation(nt[:], t5[:], AF.Tanh)
                    dh = gap.tile([128, 256], FP32, tag="dh")
                    nc.vector.tensor_sub(dh[:], h_f[:], nt[:])
                    zdh = gap.tile([128, 256], FP32, tag="zdh")
                    nc.vector.tensor_mul(zdh[:], rz[:, 256:512], dh[:])
                    h_new = statep.tile([128, 256], FP32, tag="hf")
                    nc.vector.tensor_add(h_new[:], nt[:], zdh[:])
                    nc.scalar.copy(hsv[:, :, :, t + 1],
                                   h_new[:].rearrange("p (c b) -> p c b", c=HC))
                    h_f = h_new
                nc.leave_named_scope("phaseA", _sid_pa, False)
            nc.sync.dma_start(hlast_d[:], h_f[:])
            if debug:
                nc.sync.dma_start(dbg["hs_dump"][:], hs[:])

            # ================= attention =================
            ctxT = pp.tile([128, HC * BT], BF)            # col = c*1024 + b*32 + t
            ctv = ctxT[:].rearrange("p (c b t) -> p c b t", c=HC, b=B, t=T)
            with (
                tc.tile_pool(name="attn", bufs=2) as ap_,
                tc.tile_pool(name="attn1", bufs=1) as ap1,
                tc.tile_pool(name="at_ps", bufs=1, space="PSUM") as psp,
                tc.tile_pool(name="at_ps2", bufs=2, space="PSUM") as psp2,
            ):
                _sid_at = nc.enter_named_scope("attn", False)[0]
                ident = ap1.tile([128, 128], BF)
                make_identity(nc, ident[:])
                # scores [t, s] per b; col-tiled 4 b's concurrently
                sc_ps = psp.tile([128, 512], FP32, tag="sc")
                for c in range(HC):
                    encT_t = ap_.tile([128, B * S], BF, tag="encT")
                    nc.sync.dma_start(encT_t[:], encT_d[c * 128:(c + 1) * 128, :])
                    for b in range(B):
                        g, j = b // 4, b % 4
                        # NB: start=True clears has_written for the whole
                        # bank within this partition strip, so only the
                        # FIRST b per strip may set it.
                        nc.tensor.matmul(
                            sc_ps[32 * j:32 * (j + 1), g * 64:(g + 1) * 64],
                            hsv[:, c, b, 1:TS],
                            encT_t[:, b * S:(b + 1) * S],
                            start=(c == 0 and b < 4), stop=(c == HC - 1),
                            tile_position=(0, 32 * j),
                            skip_group_check=True,
                        )
                if debug:
                    sc_sb = ap1.tile([128, 512], FP32)
                    nc.vector.tensor_copy(sc_sb[:], sc_ps[:])
                    nc.sync.dma_start(dbg["sc_dump"][:], sc_sb[:])
                scv = sc_ps[:].rearrange("p (g s) -> p g s", g=8)
                mx = ap1.tile([128, 8], FP32)
                nc.vector.reduce_max(
                    mx[:].rearrange("p (g o) -> p g o", o=1), scv, axis=AX.X)
                nmx = ap1.tile([128, 8], FP32)
                nc.vector.tensor_scalar_mul(nmx[:], mx[:], -1.0)
                wexp = ap1.tile([128, 512], BF)
                zs = ap1.tile([128, 8], FP32)
                for g in range(8):
                    nc.scalar.activation(
                        wexp[:, g * 64:(g + 1) * 64], sc_ps[:, g * 64:(g + 1) * 64],
                        AF.Exp, bias=nmx[:, g:g + 1], scale=1.0,
                        accum_out=zs[:, g:g + 1])
                rzs = ap1.tile([128, 8], FP32)
                nc.vector.reciprocal(rzs[:], zs[:])
                wn = ap1.tile([128, 512], BF)
                for g in range(8):
                    nc.vector.tensor_scalar_mul(
                        wn[:, g * 64:(g + 1) * 64],
                        wexp[:, g * 64:(g + 1) * 64], rzs[:, g:g + 1])
                # transpose w -> wT [64 (s), g*128 + j*32 + t]
                wT_ps = psp.tile([64, 1024], BF, tag="wt")
                for g in range(8):
                    nc.tensor.transpose(
                        wT_ps[:, g * 128:(g + 1) * 128],
                        wn[:, g * 64:(g + 1) * 64], ident[:])
                wT = ap1.tile([64, 1024], BF)
                nc.vector.tensor_copy(wT[:], wT_ps[:])
                if debug:
                    nc.sync.dma_start(dbg["wn_dump"][:], wn[:])
                    nc.sync.dma_start(dbg["wT_dump"][:], wT[:])

                # ctx[b] = enc_b.T @ w_b; 16 b's batched per psum flush
                for half in range(2):
                    encbs = ap_.tile([64, 16 * H], BF, tag="encbs")
                    for bl in range(16):
                        nc.sync.dma_start(
                            encbs[:, bl * H:(bl + 1) * H],
                            encN_d[half * 16 + bl, :, :])
                    for c in range(HC):
                        ctx_ps = psp2.tile([128, 512], FP32, tag="cx")
                        for bl in range(16):
                            b = half * 16 + bl
                            g, j = b // 4, b % 4
                            nc.tensor.matmul(
                                ctx_ps[:, bl * 32:(bl + 1) * 32],
                                encbs[:, bl * H + c * 128: bl * H + (c + 1) * 128],
                                wT[:, g * 128 + j * 32: g * 128 + (j + 1) * 32],
                                start=True, stop=True,
                                skip_group_check=True,
                            )
                        nc.vector.tensor_copy(
                            ctv[:, c, half * 16:(half + 1) * 16, :],
                            ctx_ps[:].rearrange("p (b t) -> p b t", t=32))

                nc.leave_named_scope("attn", _sid_at, False)
            if debug:
                nc.sync.dma_start(dbg["ctx_dump"][:], ctxT[:])

            # ================= o = tanh([hs; ctx] @ wa.T) =================
            oT = pp.tile([128, HC * BT], BF)              # col = m*1024 + bt
            with (
                tc.tile_pool(name="wa", bufs=1) as wap,
                tc.tile_pool(name="wa_ps", bufs=2, space="PSUM") as wpsp,
            ):
                _sid_wa = nc.enter_named_scope("wa", False)[0]
                wa_sb = wap.tile([128, 16 * H], BF)
                for k in range(16):
                    nc.sync.dma_start(wa_sb[:, k * H:(k + 1) * H],
                                      waT_d[k * 128:(k + 1) * 128, :])
                for m in range(HC):
                    ps0 = wpsp.tile([128, 512], FP32, tag="wps0", name="wps0")
                    ps1 = wpsp.tile([128, 512], FP32, tag="wps1", name="wps1")
                    pss = (ps0, ps1)
                    for k in range(16):
                        for n in range(2):
                            if k < HC:
                                rhs = hsv[:, k, n * 16:(n + 1) * 16, 1:TS]
                            else:
                                rhs = ctv[:, k - HC, n * 16:(n + 1) * 16, :]
                            nc.tensor.matmul(
                                pss[n][:],
                                wa_sb[:, k * H + m * 128: k * H + (m + 1) * 128],
                                rhs, start=(k == 0), stop=(k == 15),
                            )
                    for n in range(2):
                        nc.scalar.activation(
                            oT[:, m * BT + n * 512: m * BT + (n + 1) * 512],
                            pss[n][:], AF.Tanh)
                nc.leave_named_scope("wa", _sid_wa, False)

            # ================= fc + log_softmax =================
            with (
                tc.tile_pool(name="fc", bufs=2) as fcp,
                tc.tile_pool(name="fc1", bufs=1) as fc1,
                tc.tile_pool(name="fc_ps", bufs=2, space="PSUM") as fpsp,
            ):
                _sid_fc = nc.enter_named_scope("fc", False)[0]
                logits = fc1.tile([128, HC * VS], BF)     # col = j*4000 + n*500
                fcb_bc = fc1.tile([128, VS], BF)
                with tc.tile_pool(name="fcbtmp", bufs=1) as fcbp:
                    fcb_sb = fcbp.tile([1, VS], FP32)
                    nc.sync.dma_start(fcb_sb[:], fcb_d[:])
                    ones = fcbp.tile([1, 128], FP32)
                    nc.vector.memset(ones[:], 1.0)
                    for n in range(NV):
                        psb = fpsp.tile([128, 500], FP32, tag="fcb")
                        nc.tensor.matmul(psb[:], ones[:],
                                         fcb_sb[:, n * 500:(n + 1) * 500],
                                         start=True, stop=True)
                        nc.vector.tensor_copy(fcb_bc[:, n * 500:(n + 1) * 500],
                                              psb[:])

                se_acc = fc1.tile([128, 8], FP32)
                nc.vector.memset(se_acc[:], 0.0)
                for n in range(NV):
                    fwt = fcp.tile([128, HC * 500], BF, tag="fwt")
                    for k in range(HC):
                        nc.sync.dma_start(
                            fwt[:, k * 500:(k + 1) * 500],
                            fcwT_d[k * 128:(k + 1) * 128, n * 500:(n + 1) * 500])
                    for j in range(HC):
                        ps = fpsp.tile([128, 500], FP32, tag="fps")
                        for k in range(HC):
                            nc.tensor.matmul(
                                ps[:],
                                oT[:, k * BT + j * 128: k * BT + (j + 1) * 128],
                                fwt[:, k * 500:(k + 1) * 500],
                                start=(k == 0), stop=(k == HC - 1),
                            )
                        lslice = logits[:, j * VS + n * 500: j * VS + (n + 1) * 500]
                        nc.vector.tensor_add(lslice, ps[:],
                                             fcb_bc[:, n * 500:(n + 1) * 500])
                        escr = fcp.tile([128, 500], FP32, tag="escr")
                        pe = fcp.tile([128, 1], FP32, tag="pe")
                        nc.scalar.activation(escr[:], lslice, AF.Exp,
                                             accum_out=pe[:])
                        nc.vector.tensor_add(se_acc[:, j:j + 1],
                                             se_acc[:, j:j + 1], pe[:])

                if debug:
                    nc.sync.dma_start(dbg["oT_dump"][:], oT[:])
                    nc.sync.dma_start(dbg["se_dump"][:], se_acc[:])

                # global sum(exp) across vocab shards
                nc.sync.dma_start(se_loc[:], se_acc[:])
                nc.gpsimd.collective_compute(
                    "AllReduce", mybir.AluOpType.add,
                    ins=[se_loc[:]], outs=[se_glob[:]],
                    replica_groups=[list(range(NCORES))],
                )
                se_g = fc1.tile([128, 8], FP32)
                nc.sync.dma_start(se_g[:], se_glob[:])
                lse = fc1.tile([128, 8], FP32)
                nc.scalar.activation(lse[:], se_g[:], AF.Ln)
                nlse = fc1.tile([128, 8], FP32)
                nc.vector.tensor_scalar_mul(nlse[:], lse[:], -1.0)
                nc.leave_named_scope("fc", _sid_fc, False)

                _sid_ow = nc.enter_named_scope("outw", False)[0]
                for j in range(HC):
                    fout = fcp.tile([128, VS], FP32, tag="fout", name="fout")
                    for n in range(NV):
                        nc.scalar.activation(
                            fout[:, n * 500:(n + 1) * 500],
                            logits[:, j * VS + n * 500: j * VS + (n + 1) * 500],
                            AF.Identity, bias=nlse[:, j:j + 1], scale=1.0)
                    nc.sync.dma_start(out_d[j * 128:(j + 1) * 128, :], fout[:])
                nc.leave_named_scope("outw", _sid_ow, False)
    dedupe_ldweights(nc)
    split_excess_waits(nc)
    return nc


_NC_CACHE = {}


def _get_nc(t_steps=T, debug=False):
    key = (t_steps, debug)
    if key not in _NC_CACHE:
        _NC_CACHE[key] = build_nc(t_steps, debug)
    return _NC_CACHE[key]


def host_prep(inputs):
    """Build the per-core input maps from the full inputs."""
    target = np.asarray(inputs["target"])
    enc_h = np.asarray(inputs["encoder_hidden"], np.float32)
    enc_o = np.asarray(inputs["encoder_outputs"], np.float32)
    emb = np.asarray(inputs["emb"], np.float32)
    w_ih = np.asarray(inputs["w_ih"], np.float32)
    w_hh = np.asarray(inputs["w_hh"], np.float32)
    b_ih = np.asarray(inputs["b_ih"], np.float32)
    b_hh = np.asarray(inputs["b_hh"], np.float32)
    wa = np.asarray(inputs["wa"], np.float32)
    fc_w = np.asarray(inputs["fc_w"], np.float32)
    fc_b = np.asarray(inputs["fc_b"], np.float32)

    in_ids = np.concatenate(
        [np.full((B, 1), 1, target.dtype), target[:, :T - 1]], axis=1)
    x_emb = emb[in_ids]                               # [B,T,E]
    xembT = np.ascontiguousarray(
        x_emb.transpose(2, 1, 0).reshape(E, BT)).astype(BF16)  # col = t*32+b

    whhT = np.ascontiguousarray(w_hh.T).astype(BF16)  # [H, 3H]
    wihT = np.ascontiguousarray(w_ih.T).astype(BF16)  # [E, 3H]

    brz_v = (b_ih[:2 * H] + b_hh[:2 * H]).reshape(16, 128).T.copy()   # [128,16]
    bin_v = b_ih[2 * H:].reshape(8, 128).T.copy()                     # [128,8]
    bhn_v = b_hh[2 * H:].reshape(8, 128).T.copy()                     # [128,8]

    h0 = enc_h[0]                                      # [B,H]
    h0T = np.ascontiguousarray(
        h0.T.reshape(HC, 128, B).transpose(1, 0, 2).reshape(128, 256))

    encT = np.ascontiguousarray(
        enc_o.transpose(2, 0, 1).reshape(H, B * S)).astype(BF16)
    encN = enc_o.astype(BF16)
    waT = np.ascontiguousarray(wa.T).astype(BF16)      # [2H, H]

    common = {
        "whhT": whhT, "wihT": wihT, "xembT": xembT,
        "brz": brz_v.astype(np.float32), "bin": bin_v.astype(np.float32),
        "bhn": bhn_v.astype(np.float32), "h0T": h0T.astype(np.float32),
        "encT": encT, "encN": encN, "waT": waT,
    }
    in_maps = []
    for c in range(NCORES):
        v0, v1 = c * VS, (c + 1) * VS
        m = dict(common)
        m["fcwT"] = np.ascontiguousarray(fc_w[v0:v1].T).astype(BF16)
        m["fcb"] = fc_b[v0:v1].reshape(1, VS).astype(np.float32)
        in_maps.append(m)
    return in_maps


def run_spmd(in_maps, t_steps=T, trace=False, debug=False, **kw):
    from concourse.bass_utils import run_bass_kernel_spmd
    nc = _get_nc(t_steps, debug)
    return run_bass_kernel_spmd(nc, in_maps, list(range(NCORES)), trace=trace, **kw)


def assemble(results):
    out_full = np.concatenate([results[c]["out"] for c in range(NCORES)], axis=1)
    decoder_outputs = out_full.reshape(B, T, V)
    hl = results[0]["hlast"]                            # [128, 256]
    h_last = hl.reshape(128, HC, B).transpose(2, 1, 0).reshape(B, H)
    return decoder_outputs, h_last[None]


def kernel(**inputs):
    in_maps = host_prep(inputs)
    res = run_spmd(in_maps)
    return assemble(res.results)
